# revision 13
# baseline (speedup 1.0000x reference)
"""CoAttention Trainium2 Bass kernel.

Problem (per batch b):
  v1 = text @ W1                               [T,1]
  v2 = img @ W2                                [I,1]
  v3 = (text * W3^T) @ img^T                   [T,I]
  v  = v1 + v2^T + v3 + bias                   [T,I]
  A_img  = softmax(v, axis=I)
  A_text = softmax(max(v, axis=I), axis=T)
  text_re = A_text^T @ text                    [1,D]
  img_re  = A_img @ img                        [T,D]
  G = concat([text, img_re, text*img_re, text*text_re], -1)   [T,4D]

Sharding: data-parallel over batch B=32 across 8 cores (4 batches/core),
weights replicated.

Device algorithm (all in transposed [I,T] layout so A_img never needs a
transpose):
  R[d,i]   = W3[d]*imgT[d,i] + W1[d]            (folds v1 into the matmul)
  vT[i,t]  = sum_d R[d,i]*textT[d,t]            (PE, bf16)
  expT     = exp(vT + (v2[i]+bias))             (ACT, bias is per-partition)
  s[t]     = sum_i expT  (PE matmul w/ ones);  img_re = expT^T @ img (PE)
  m'[t]    = max_i expT  (elementwise max over i-tiles + PE transpose +
             free-dim reduce);  A_text = m'/sum(m')  (exp is monotone)
  text_re  = (1/Z) * sum_t m'[t]*text[t,:]      (PE, rank-1 style)
  G blocks assembled on-chip, streamed out per 128-row tile.
"""

import numpy as np
import ml_dtypes

import concourse.bass as bass
import concourse.mybir as mybir
from concourse import bacc
from concourse.tile import TileContext
from concourse.bass_utils import run_bass_kernel_spmd

B, T, I, D = 32, 1024, 512, 512
N_CORES = 8
BPC = B // N_CORES  # batches per core

F32 = mybir.dt.float32
BF16 = mybir.dt.bfloat16

_AF = mybir.ActivationFunctionType
_OP = mybir.AluOpType


def _build_bass(repeats=1):
    nc = bacc.Bacc()

    text_in = nc.dram_tensor("text_in", [BPC, T, D], F32, kind="ExternalInput")
    img_in = nc.dram_tensor("img_in", [BPC, I, D], F32, kind="ExternalInput")
    # host-folded weight constants, packed so each loads with ONE dma
    # consts_f cols: 0:4 w3c | 4:8 w1c | 8:9 bias | 9:137 ident_f | 137:138 ones_f
    consts_f = nc.dram_tensor("consts_f", [128, 138], F32, kind="ExternalInput")
    # consts_b cols: 0:4 w2 | 4:132 ident_b | 132:133 ones_b
    consts_b = nc.dram_tensor("consts_b", [128, 133], BF16, kind="ExternalInput")

    g_out = nc.dram_tensor("g_out", [BPC, T, 4 * D], F32, kind="ExternalOutput")

    NT = T // 128  # 8 t-tiles
    NI = I // 128  # 4 i-tiles
    NDC = D // 128  # 4 d-chunks

    with TileContext(nc) as tc:
        with (
            tc.tile_pool(name="consts", bufs=1) as cpool,
            tc.tile_pool(name="big", bufs=2) as bpool,
            tc.tile_pool(name="gbufs", bufs=4) as gpool,
            tc.tile_pool(name="small", bufs=3) as spool,
            tc.tile_pool(name="ps_big", bufs=4, space="PSUM") as ps_big,
            tc.tile_pool(name="ps_small", bufs=4, space="PSUM") as ps_small,
        ):
            c_f = cpool.tile([128, 138], F32)
            nc.sync.dma_start(c_f, consts_f[:, :])
            c_b = cpool.tile([128, 133], BF16)
            nc.sync.dma_start(c_b, consts_b[:, :])
            c_w3 = c_f[:, 0:4]
            c_w1 = c_f[:, 4:8]
            c_bias = c_f[:, 8:9]
            c_idf = c_f[:, 9:137]
            c_onesf = c_f[:, 137:138]
            c_w2 = c_b[:, 0:4]
            c_idb = c_b[:, 4:132]
            c_onesb = c_b[:, 132:133]

            for b in [bb for _ in range(repeats) for bb in range(BPC)]:
                # ---- loads ----
                # text rows t = n*128 + p  ->  [p, n, d]
                text_sb = bpool.tile([128, NT, D], F32, tag="text_sb")
                nc.sync.dma_start(
                    text_sb, text_in[b].rearrange("(n p) d -> p n d", p=128)
                )
                # img rows i = m*128 + p -> [p, m, d], cast f32->bf16 in DMA
                img_bf = bpool.tile([128, NI, D], BF16, tag="img_bf")
                nc.gpsimd.dma_start(
                    img_bf, img_in[b].rearrange("(m p) d -> p m d", p=128)
                )

                # ---- imgT (PE transpose) -> Rt = W3*imgT + W1, imgT_bf ----
                rt_bf = bpool.tile([128, NDC, I], BF16, tag="rt_bf")
                imgT_bf = bpool.tile([128, NDC, I], BF16, tag="imgT_bf")
                for c in range(NDC):
                    ps_it = ps_big.tile([128, I], BF16, tag="pb", name="ps_it")
                    for m in range(NI):
                        nc.tensor.transpose(
                            ps_it[:, m * 128 : (m + 1) * 128],
                            img_bf[:, m, c * 128 : (c + 1) * 128],
                            c_idb,
                        )
                    nc.vector.tensor_scalar(
                        rt_bf[:, c, :],
                        ps_it,
                        c_w3[:, c : c + 1],
                        c_w1[:, c : c + 1],
                        _OP.mult,
                        _OP.add,
                    )
                    nc.scalar.activation(imgT_bf[:, c, :], ps_it, _AF.Copy)

                # ---- text_bf (bf16 cast, for text_re matmul rhs) ----
                text_bf = bpool.tile([128, NT, D], BF16, tag="text_bf")
                nc.vector.tensor_copy(text_bf, text_sb)

                # ---- textT (PE transpose, f32 -> copy-cast bf16) ----
                textT_bf = bpool.tile([128, NDC, T], BF16, tag="textT_bf")
                for c in range(NDC):
                    for ng in range(2):
                        ps_tt = ps_big.tile([128, 512], F32, tag="pb", name="ps_tt")
                        for k in range(4):
                            n = ng * 4 + k
                            nc.tensor.transpose(
                                ps_tt[:, k * 128 : (k + 1) * 128],
                                text_sb[:, n, c * 128 : (c + 1) * 128],
                                c_idf,
                            )
                        nc.scalar.activation(
                            textT_bf[:, c, ng * 512 : (ng + 1) * 512], ps_tt, _AF.Copy
                        )

                # ---- v2 = img @ W2 (tiny matmuls), v2b = v2 + bias ----
                ps_v2 = ps_small.tile([128, NI], F32, tag="ps", name="ps_v2")
                for m in range(NI):
                    for c in range(NDC):
                        nc.tensor.matmul(
                            ps_v2[:, m : m + 1],
                            imgT_bf[:, c, m * 128 : (m + 1) * 128],
                            c_w2[:, c : c + 1],
                            start=(c == 0),
                            stop=(c == NDC - 1),
                        )
                v2b = spool.tile([128, NI], F32, tag="v2b")
                nc.scalar.activation(v2b, ps_v2, _AF.Identity, bias=c_bias, scale=1.0)

                # ---- vT = R^T @ textT ; expT = exp(vT + v2b) ----
                expT_bf = bpool.tile([128, NI, T], BF16, tag="expT_bf")
                for m in range(NI):
                    for t2 in range(2):
                        ps_vt = ps_big.tile([128, 512], F32, tag="pb", name="ps_vt")
                        for c in range(NDC):
                            nc.tensor.matmul(
                                ps_vt,
                                rt_bf[:, c, m * 128 : (m + 1) * 128],
                                textT_bf[:, c, t2 * 512 : (t2 + 1) * 512],
                                start=(c == 0),
                                stop=(c == NDC - 1),
                            )
                        nc.scalar.activation(
                            expT_bf[:, m, t2 * 512 : (t2 + 1) * 512],
                            ps_vt,
                            _AF.Exp,
                            bias=v2b[:, m : m + 1],
                            scale=1.0,
                        )

                # ---- m'[t] = max_i expT ----
                mx01 = spool.tile([128, T], BF16, tag="mx01")
                mx23 = spool.tile([128, T], BF16, tag="mx23")
                m8 = spool.tile([128, T], BF16, tag="m8")
                nc.vector.tensor_max(mx01, expT_bf[:, 0, :], expT_bf[:, 1, :])
                nc.vector.tensor_max(mx23, expT_bf[:, 2, :], expT_bf[:, 3, :])
                nc.vector.tensor_max(m8, mx01, mx23)
                mprime = spool.tile([128, NT], BF16, tag="mprime")
                for n in range(NT):
                    ps_mt = ps_big.tile([128, 128], BF16, tag="pb", name="ps_mt")
                    nc.tensor.transpose(ps_mt, m8[:, n * 128 : (n + 1) * 128], c_idb)
                    nc.vector.reduce_max(
                        mprime[:, n : n + 1], ps_mt, axis=mybir.AxisListType.X
                    )

                # ---- Z = sum_t m', rZ = 1/Z ----
                ps_z = ps_small.tile([1, 1], F32, tag="ps", name="ps_z")
                for n in range(NT):
                    nc.tensor.matmul(
                        ps_z,
                        mprime[:, n : n + 1],
                        c_onesb,
                        start=(n == 0),
                        stop=(n == NT - 1),
                    )
                rz = spool.tile([1, 1], F32, tag="rz")
                nc.vector.reciprocal(rz, ps_z)

                # ---- text_re row: tre[1,d] = sum_t m'[t] text[t,d] (m' stationary) ----
                ps_trr = ps_small.tile([1, 512], F32, tag="ps", name="ps_trr")
                for n in range(NT):
                    nc.tensor.matmul(
                        ps_trr,
                        mprime[:, n : n + 1],
                        text_bf[:, n, :],
                        start=(n == 0),
                        stop=(n == NT - 1),
                    )
                trerow = spool.tile([1, 512], F32, tag="trerow")
                nc.scalar.activation(trerow, ps_trr, _AF.Copy, scale=rz)
                bcast = spool.tile([128, 512], F32, tag="bcast")
                nc.gpsimd.partition_broadcast(bcast, trerow)

                # ---- store text block of G (pure copy) ----
                nc.sync.dma_start(
                    g_out[b].rearrange("(n p) g -> p n g", p=128)[:, :, 0:D], text_sb
                )

                # ---- per t-tile: img_re, s, G assembly, store ----
                for n in range(NT):
                    ps_ir = ps_big.tile([128, D], F32, tag="pb", name="ps_ir")
                    for m in range(NI):
                        nc.tensor.matmul(
                            ps_ir,
                            expT_bf[:, m, n * 128 : (n + 1) * 128],
                            img_bf[:, m, :],
                            start=(m == 0),
                            stop=(m == NI - 1),
                        )
                    ps_s = ps_small.tile([128, 1], F32, tag="ps", name="ps_s")
                    for m in range(NI):
                        nc.tensor.matmul(
                            ps_s,
                            expT_bf[:, m, n * 128 : (n + 1) * 128],
                            c_onesb,
                            start=(m == 0),
                            stop=(m == NI - 1),
                        )
                    rs = spool.tile([128, 1], F32, tag="rs")
                    nc.vector.reciprocal(rs, ps_s)

                    gbuf = gpool.tile([128, 3 * D], F32, tag="gbuf")
                    # img_re (normalized)
                    nc.scalar.activation(gbuf[:, 0:D], ps_ir, _AF.Copy, scale=rs)
                    # text * img_re
                    nc.vector.scalar_tensor_tensor(
                        gbuf[:, D : 2 * D],
                        ps_ir,
                        rs,
                        text_sb[:, n, :],
                        _OP.mult,
                        _OP.mult,
                    )
                    # text * text_re (on GPSIMD -- otherwise idle engine)
                    nc.gpsimd.tensor_mul(
                        gbuf[:, 2 * D : 3 * D], text_sb[:, n, :], bcast
                    )
                    nc.sync.dma_start(
                        g_out[b, n * 128 : (n + 1) * 128, D : 4 * D], gbuf
                    )

    nc.compile()
    return nc


_cache = {}


def _get_nc(repeats=1):
    key = f"nc{repeats}"
    if key not in _cache:
        _cache[key] = _build_bass(repeats)
    return _cache[key]


def _host_consts(W1, W2, W3, bias):
    w3c = W3[:, 0].reshape(4, 128).T.astype(np.float32)
    w1c = W1[:, 0].reshape(4, 128).T.astype(np.float32)
    w2c = W2[:, 0].reshape(4, 128).T.astype(np.float32)
    bias_col = np.full((128, 1), np.float32(bias[0]), dtype=np.float32)
    ident = np.eye(128, dtype=np.float32)
    ones = np.ones((128, 1), dtype=np.float32)
    consts_f = np.ascontiguousarray(
        np.concatenate([w3c, w1c, bias_col, ident, ones], axis=1, dtype=np.float32)
    )
    consts_b = np.ascontiguousarray(
        np.concatenate([w2c, ident, ones], axis=1).astype(ml_dtypes.bfloat16)
    )
    return dict(consts_f=consts_f, consts_b=consts_b)


def _run(inputs, trace=False, trace_kwargs=None):
    text = np.ascontiguousarray(np.asarray(inputs["text"], dtype=np.float32))
    img = np.ascontiguousarray(np.asarray(inputs["img"], dtype=np.float32))
    consts = _host_consts(
        np.asarray(inputs["W1"], dtype=np.float32),
        np.asarray(inputs["W2"], dtype=np.float32),
        np.asarray(inputs["W3"], dtype=np.float32),
        np.asarray(inputs["bias"], dtype=np.float32),
    )
    nc = _get_nc()
    in_maps = []
    for core in range(N_CORES):
        sl = slice(core * BPC, (core + 1) * BPC)
        in_maps.append(
            dict(
                text_in=np.ascontiguousarray(text[sl]),
                img_in=np.ascontiguousarray(img[sl]),
                **consts,
            )
        )
    kwargs = {}
    if trace:
        kwargs["trace"] = True
        if trace_kwargs:
            kwargs["trace_kwargs"] = trace_kwargs
    res = run_bass_kernel_spmd(nc, in_maps, core_ids=list(range(N_CORES)), **kwargs)
    out = np.concatenate([r["g_out"] for r in res.results], axis=0)
    return out, res


def kernel(**inputs) -> np.ndarray:
    out, _ = _run(inputs, trace=False)
    return out
